# revision 8
# baseline (speedup 1.0000x reference)
"""Trainium2 Bass kernel for nn_DetectorHelper (seq2seq LSTM anomaly detector).

Architecture: encoder LSTM over T=1024 steps -> decoder LSTM over reversed
sequence emitting a linear projection of the hidden state before each cell
update. Data-parallel over the batch axis: 8 NeuronCores x 16 batch rows.

Per-core design (state-stationary matmuls, batch-major cell math):
  - gates [16, 4H] accumulate in PSUM from 12 fp32r matmuls per step
    (x-sliver + two hT K-tiles, per gate-chunk g -> if -> o so tanh(g) is
    ready earliest and sigmoid(o), needed last, finishes last).
  - fp32r (TF32-like rounded fp32) streams weight panels through the PE at
    1 cycle/row; end-to-end relative error ~1.7e-4 (LSTM recurrence is
    contractive, so the rounding error does not accumulate).
  - cell update on ACT (sigmoid/tanh) + DVE; h is transposed back to [H, 16]
    via two PE transposes for the next step's stationary operand.
  - For_i blocks of U=64 steps; x slivers staged by per-step DMA into an
    8-slot SBUF ring (matmul lhsT requires static offsets).
"""

import sys

sys.path.insert(0, "/opt/trn_rl_repo")

from contextlib import ExitStack

import numpy as np

B = 16      # batch rows per core
F = 64      # feature dim
H = 256     # hidden dim
G = 4 * H   # gate dim
T = 1024
U = 64      # timesteps per For_i body
XS = 8      # x staging ring slots
N_CORES = 8

_CACHE = {}


def build_kernel(repeat=1, T_=None, unroll=False):
    import concourse.bass as bass
    import concourse.tile as tile
    from concourse import bacc, mybir

    F32 = mybir.dt.float32
    F32R = mybir.dt.float32r
    Tk = T if T_ is None else T_
    NB = Tk // U

    nc = bacc.Bacc("TRN2", target_bir_lowering=False, debug=False,
                   num_devices=N_CORES)

    xte_d = nc.dram_tensor("xte", [F + 1, Tk * B], F32, kind="ExternalInput").ap()
    xtd_d = nc.dram_tensor("xtd", [F + 1, Tk * B], F32, kind="ExternalInput").ap()
    wih_e_d = nc.dram_tensor("wih_e", [F + 1, G], F32, kind="ExternalInput").ap()
    whh_e_d = nc.dram_tensor("whh_e", [128, 2 * G], F32, kind="ExternalInput").ap()
    wih_d_d = nc.dram_tensor("wih_d", [F + 1, G], F32, kind="ExternalInput").ap()
    whh_d_d = nc.dram_tensor("whh_d", [128, 2 * G], F32, kind="ExternalInput").ap()
    wout_d = nc.dram_tensor("wout", [128, 2 * F], F32, kind="ExternalInput").ap()
    bout_d = nc.dram_tensor("bout", [B, F], F32, kind="ExternalInput").ap()
    ident_d = nc.dram_tensor("ident", [B, B], F32, kind="ExternalInput").ap()
    out_d = nc.dram_tensor("out", [B, Tk * F], F32, kind="ExternalOutput").ap()

    with tile.TileContext(nc) as tc, ExitStack() as ctx:
        wpool = ctx.enter_context(tc.tile_pool(name="wpool", bufs=1))
        wih_e = wpool.tile([F + 1, G], F32R, name="wih_e_sb")
        whh_e = wpool.tile([128, 2 * G], F32R, name="whh_e_sb")
        wih_d = wpool.tile([F + 1, G], F32R, name="wih_d_sb")
        whh_d = wpool.tile([128, 2 * G], F32R, name="whh_d_sb")
        wout = wpool.tile([128, 2 * F], F32R, name="wout_sb")
        bout = wpool.tile([B, F], F32, name="bout_sb")
        ident = wpool.tile([B, B], F32, name="ident_sb")
        nc.sync.dma_start(bout[:], bout_d[:])
        nc.sync.dma_start(ident[:], ident_d[:])
        # fp32r matmul operands must come from a rounding producer, so DMA to
        # fp32 staging and round-copy on DVE.
        for sb, dr in [(wih_e, wih_e_d), (whh_e, whh_e_d), (wih_d, wih_d_d),
                       (whh_d, whh_d_d), (wout, wout_d)]:
            stg = wpool.tile(list(sb.shape), F32, name="wstg", tag="wstg", bufs=2)
            nc.sync.dma_start(stg[:], dr[:])
            nc.vector.tensor_copy(sb[:], stg[:])

        # persistent state, parity ping-pong: step j reads half p=j%2, writes 1-p
        hT = wpool.tile([128, 64], F32R, name="hT_sb")
        cst = wpool.tile([B, 2 * H], F32, name="c_sb")
        zinit = wpool.tile([128, 32], F32, name="zinit_sb")
        nc.vector.memset(zinit[:], 0.0)

        xstage = wpool.tile([F + 1, XS * B], F32, name="xstage_sb")
        xstager = wpool.tile([F + 1, XS * B], F32R, name="xstager_sb")

        gpool = ctx.enter_context(tc.tile_pool(name="gpool", bufs=2, space="PSUM"))
        tpool = ctx.enter_context(tc.tile_pool(name="tpool", bufs=2, space="PSUM"))
        apool = ctx.enter_context(tc.tile_pool(name="apool", bufs=3))
        cpool = ctx.enter_context(tc.tile_pool(name="cpool", bufs=3))
        spool = ctx.enter_context(tc.tile_pool(name="spool", bufs=2))

        SIG = mybir.ActivationFunctionType.Sigmoid
        TANH = mybir.ActivationFunctionType.Tanh

        def step(blk, j, xsrc_d, wih, whh, dec_ostage=None):
            p = j % 2
            h_prev = hT[:, 32 * p:32 * p + 32]
            h_next = hT[:, 32 * (1 - p):32 * (1 - p) + 32]
            c_prev = cst[:, H * p:H * p + H]
            c_next = cst[:, H * (1 - p):H * (1 - p) + H]
            sl = B * (j % XS)
            xslot = xstage[:, sl:sl + B]
            xslotr = xstager[:, sl:sl + B]

            nc.sync.dma_start(xslot, xsrc_d[:, bass.ts(blk * U + j, B)])
            nc.gpsimd.tensor_copy(xslotr, xslot)

            g_ps = gpool.tile([B, G], F32, name="g_ps")

            def chunk_mms(cs, cn):
                nc.tensor.matmul(g_ps[:, cs:cs + cn], xslotr, wih[:, cs:cs + cn],
                                 start=True, stop=False)
                nc.tensor.matmul(g_ps[:, cs:cs + cn], h_prev[:, 0:16],
                                 whh[:, cs:cs + cn], start=False, stop=False)
                nc.tensor.matmul(g_ps[:, cs:cs + cn], h_prev[:, 16:32],
                                 whh[:, G + cs:G + cs + cn], start=False, stop=True)

            # gate layout [i f g o]; compute g first (feeds the longest
            # dependency path), o last (only needed for the final h multiply)
            chunk_mms(512, 256)   # g
            chunk_mms(0, 256)     # i
            chunk_mms(256, 256)   # f
            chunk_mms(768, 256)   # o
            if dec_ostage is not None:
                # after the gate MMs so the in-order PE starts the
                # chain-critical h-matmuls first
                ostage, col = dec_ostage
                o_ps = tpool.tile([B, F], F32, name="o_ps", tag="tops")
                nc.tensor.matmul(o_ps[:], h_prev[:, 0:16], wout[:, 0:F],
                                 start=True, stop=False)
                nc.tensor.matmul(o_ps[:], h_prev[:, 16:32], wout[:, F:2 * F],
                                 start=False, stop=True)
                nc.vector.tensor_add(ostage[:, col:col + F], o_ps[:], bout[:])

            gact = apool.tile([B, G], F32, name="gact")
            nc.scalar.activation(gact[:, 512:768], g_ps[:, 512:768], TANH)
            nc.scalar.activation(gact[:, 0:H], g_ps[:, 0:H], SIG)
            ig = cpool.tile([B, H], F32, name="ig")
            nc.vector.tensor_mul(ig[:], gact[:, 0:H], gact[:, 2 * H:3 * H])
            nc.scalar.activation(gact[:, H:2 * H], g_ps[:, H:2 * H], SIG)
            fc = cpool.tile([B, H], F32, name="fc")
            nc.vector.tensor_mul(fc[:], gact[:, H:2 * H], c_prev)
            nc.vector.tensor_add(c_next, ig[:], fc[:])
            nc.scalar.activation(gact[:, 768:1024], g_ps[:, 768:1024], SIG)
            tch = cpool.tile([B, H], F32, name="tch")
            nc.scalar.activation(tch[:], c_next, TANH)
            h_bm = cpool.tile([B, H], F32, name="h_bm")
            nc.vector.tensor_mul(h_bm[:], gact[:, 3 * H:4 * H], tch[:])

            t_ps = tpool.tile([128, 32], F32, name="t_ps", tag="tops")
            nc.tensor.transpose(t_ps[:, 0:16], h_bm[:, 0:128], ident[:])
            nc.tensor.transpose(t_ps[:, 16:32], h_bm[:, 128:256], ident[:])
            nc.vector.tensor_copy(h_next, t_ps[:])

        def enc_block(blk):
            for j in range(U):
                step(blk, j, xte_d, wih_e, whh_e)

        def dec_block(blk):
            ostage = spool.tile([B, U * F], F32, name="ostage")
            for j in range(U):
                # decoder step s emits the projection of h BEFORE the
                # update; outputs land reversed within the block (col
                # U-1-j), the block at t-range [T-(blk+1)U, T-blk*U)
                step(blk, j, xtd_d, wih_d, whh_d,
                     dec_ostage=(ostage, (U - 1 - j) * F))
            nc.sync.dma_start(out_d[:, bass.ts((NB - 1) - blk, U * F)],
                              ostage[:])

        def loop_blocks(body):
            if unroll:
                for blk in range(NB):
                    body(blk)
            else:
                with tc.For_i(0, NB) as blk:
                    body(blk)

        def run_once():
            nc.vector.tensor_copy(hT[:, 0:32], zinit[:])
            nc.vector.memset(cst[:, 0:H], 0.0)
            loop_blocks(enc_block)
            loop_blocks(dec_block)

        if repeat > 1:
            with tc.For_i(0, repeat):
                run_once()
        else:
            run_once()

    nc.compile()
    return nc


def host_prep(ts_batch, W_ih_enc, W_hh_enc, b_enc, W_ih_dec, W_hh_dec, b_dec,
              W_out, b_out):
    def prep_w(W_ih, W_hh, b):
        wihT = np.ascontiguousarray(np.asarray(W_ih, np.float32).T)      # [F, G]
        wih_aug = np.concatenate([wihT, np.asarray(b, np.float32)[None, :]], 0)
        whhT = np.asarray(W_hh, np.float32).T                            # [H, G]
        whh_pack = np.concatenate([whhT[:128], whhT[128:]], 1)           # [128, 2G]
        return np.ascontiguousarray(wih_aug), np.ascontiguousarray(whh_pack)

    wih_e, whh_e = prep_w(W_ih_enc, W_hh_enc, b_enc)
    wih_d, whh_d = prep_w(W_ih_dec, W_hh_dec, b_dec)
    woutT = np.asarray(W_out, np.float32).T
    wout_pack = np.ascontiguousarray(np.concatenate([woutT[:128], woutT[128:]], 1))
    bout_b = np.ascontiguousarray(
        np.broadcast_to(np.asarray(b_out, np.float32)[None, :], (B, F)))
    ident = np.eye(B, dtype=np.float32)

    ts = np.asarray(ts_batch, np.float32)
    in_maps = []
    for d in range(N_CORES):
        tsl = ts[d * B:(d + 1) * B]                       # [16, T, F]
        xte = np.empty((F + 1, T * B), np.float32)
        xte[:F] = tsl.transpose(2, 1, 0).reshape(F, T * B)  # col = t*16 + b
        xte[F] = 1.0
        xtd = np.ascontiguousarray(
            xte.reshape(F + 1, T, B)[:, ::-1, :].reshape(F + 1, T * B))
        in_maps.append({
            "xte": np.ascontiguousarray(xte), "xtd": xtd,
            "wih_e": wih_e, "whh_e": whh_e,
            "wih_d": wih_d, "whh_d": whh_d,
            "wout": wout_pack, "bout": bout_b, "ident": ident,
        })
    return in_maps


def unpack_out(res_list):
    outs = [r["out"].reshape(B, T, F) for r in res_list]
    return np.ascontiguousarray(np.concatenate(outs, 0))


def kernel(ts_batch, W_ih_enc, W_hh_enc, b_enc, W_ih_dec, W_hh_dec, b_dec,
           W_out, b_out):
    from concourse.bass_utils import run_bass_kernel_spmd

    if "nc" not in _CACHE:
        _CACHE["nc"] = build_kernel()
    nc = _CACHE["nc"]
    in_maps = host_prep(ts_batch, W_ih_enc, W_hh_enc, b_enc, W_ih_dec,
                        W_hh_dec, b_dec, W_out, b_out)
    res = run_bass_kernel_spmd(nc, in_maps, core_ids=list(range(N_CORES)))
    return unpack_out([r for r in res.results])


if __name__ == "__main__":
    rng = np.random.default_rng(0)
    demo = {
        "ts_batch": rng.standard_normal((128, T, F), dtype=np.float32),
        "W_ih_enc": rng.standard_normal((G, F), dtype=np.float32) * 0.06,
        "W_hh_enc": rng.standard_normal((G, H), dtype=np.float32) * 0.06,
        "b_enc": rng.standard_normal(G).astype(np.float32) * 0.06,
        "W_ih_dec": rng.standard_normal((G, F), dtype=np.float32) * 0.06,
        "W_hh_dec": rng.standard_normal((G, H), dtype=np.float32) * 0.06,
        "b_dec": rng.standard_normal(G).astype(np.float32) * 0.06,
        "W_out": rng.standard_normal((F, H), dtype=np.float32) * 0.06,
        "b_out": rng.standard_normal(F).astype(np.float32) * 0.06,
    }
    out = kernel(**demo)
    print("kernel output", out.shape, out.dtype, float(np.abs(out).max()))
